# revision 17
# baseline (speedup 1.0000x reference)
"""Trainium2 Bass kernel for nn_GCNFN (2-layer GAT + mean-pool + MLP head).

Strategy (8 NeuronCores, graph-parallel by destination node):
  * Nodes are sharded 6250/core (padded 6272). Edges (with self-loops) are
    sorted by dst and assigned to the core owning the dst node.
  * Per layer: node phase computes h_ext = x @ [W | W@att_l | W@att_r] for the
    local shard; h (bf16) and a_src (f32) are AllGathered; a_dst stays local.
  * Edge phase per dst-tile of 128 nodes: h[src] rows are fetched with
    dma_gather (bf16, 512B rows; src ids split into <32768 / >=32768 buckets
    for int16 indices), per-edge a_src/a_dst scalars with 4-byte indirect
    DMAs.  exp(leaky_relu(a_src+a_dst)) is computed compactly, expanded into a
    one-hot-times-exp matrix S^T [128 edges, 128 dst] with a single fused
    tensor_scalar op per 128-edge chunk, and the segment softmax-weighted sum
    becomes PE matmuls accumulated in PSUM (plus a ones-column matmul for the
    denominators).
  * Mean-pool partial sums are computed with host-known one-hot matmuls and
    AllReduced; the tiny MLP head + log_softmax run redundantly on every core.
"""

import sys

sys.path.insert(0, "/opt/trn_rl_repo")

import numpy as np
import ml_dtypes

BF16 = ml_dtypes.bfloat16

# ---- problem constants (hardcoded) ----
N = 50000
E0 = 800000
G = 256
F_IN = 310
C = 256
HID = 128
NCLS = 2
W = 8            # cores
NSH = 6250       # nodes per core
NSHP = 6272      # padded (49*128)
NP = NSHP * W    # 50176 padded global rows
P = 128
TPC = 49         # dst tiles per core
SBT = 2          # tiles per superblock
NSB = 25         # superblocks (50 tile slots, 1 dummy)
HW_ = 384        # gathered row width (bf16 units): 256 h + 2 a_src(f32) + pad
TPC_PAD = NSB * SBT
SPLIT = 32768    # int16 gather index limit

LAM = 1.0507009873554805
ALPHA_SELU = 1.6732632423543772
LA = LAM * ALPHA_SELU

_CACHE = {}
DEBUG_FLAGS = set()  # {"no_gather","no_indirect","no_edge","no_coll"}


def _edge_prep(edge_index):
    """Host-side edge preprocessing -> per-core index streams."""
    src = np.concatenate([edge_index[0], np.arange(N)]).astype(np.int64)
    dst = np.concatenate([edge_index[1], np.arange(N)]).astype(np.int64)
    order = np.argsort(dst, kind="stable")
    src_s = src[order]
    dst_s = dst[order]
    srcp = (src_s // NSH) * NSHP + (src_s % NSH)   # padded global src ids
    core_of = dst_s // NSH
    dl_sh = dst_s % NSH                             # local dst id within shard
    tile_of = dl_sh // P
    dloc = dl_sh % P                                # dst slot within tile
    low = srcp < SPLIT

    key = core_of * TPC + tile_of
    nlow = np.bincount(key[low], minlength=W * TPC).reshape(W, TPC)
    nhigh = np.bincount(key[~low], minlength=W * TPC).reshape(W, TPC)
    CLn = int(np.ceil(nlow.max() / P))
    CHn = int(np.ceil(nhigh.max() / P))
    CT = CLn + CHn
    M = SBT * CT

    idxlo = np.zeros((W, NSB, P, SBT * CLn * 8), np.int16)
    idxhi = np.zeros((W, NSB, P, SBT * CHn * 8), np.int16)
    adidx = np.zeros((W, NSB, P, M * 8), np.int16)
    dstloc = np.full((W, NSB, P, M), 255.0, np.float32)

    for c in range(W):
        msk_c = core_of == c
        for t in range(TPC):
            sb, j = divmod(t, SBT)
            msk_t = msk_c & (tile_of == t)
            for is_low, CB, chunk0, idxarr, vals_off in (
                (True, CLn, j * CLn, idxlo, 0),
                (False, CHn, SBT * CLn + j * CHn, idxhi, SPLIT),
            ):
                m = msk_t & (low if is_low else ~low)
                vals = srcp[m] - vals_off          # gather indices (int16 range)
                dl = dloc[m]
                asv = srcp[m]                      # a_src gather ids (global padded)
                adv = dl_sh[m]                     # a_dst gather ids (local)
                n = len(vals)
                assert n <= CB * P
                q = np.arange(n)
                # gather index list (wrapped [i%16, i//16], replicated x8)
                Lpos = (j if is_low else j) * CB * P + q
                rows = (Lpos % 16)
                cols = Lpos // 16
                for g in range(8):
                    idxarr[c, sb, g * 16 + rows, cols] = vals.astype(np.int16)
                # gather padding -> index 0 (valid dummy row)
                # (array already zero-initialized)
                pp = q % P
                ch = chunk0 + q // P
                dstloc[c, sb, pp, ch] = dl.astype(np.float32)
                # a_dst gather list position = ch*128 + pp (call-chunk order)
                Qpos = ch * P + pp
                for g in range(8):
                    adidx[c, sb, g * 16 + Qpos % 16, Qpos // 16] = adv.astype(np.int16)

    return dict(CL=CLn, CH=CHn, CT=CT, M=M, idxlo=idxlo, idxhi=idxhi,
                dstloc=dstloc, adidx=adidx)


def _build(CLn, CHn):
    import concourse.bass as bass
    import concourse.bacc as bacc
    import concourse.tile as tile
    import concourse.mybir as mybir
    from concourse.masks import make_identity

    dt = mybir.dt
    op = mybir.AluOpType
    act = mybir.ActivationFunctionType

    CT = CLn + CHn
    M = SBT * CT

    nc = bacc.Bacc("TRN2", target_bir_lowering=False, debug=False,
                   enable_asserts=False, num_devices=W, num_swdge_queues=4)

    # ---------------- I/O ----------------
    xT_in = nc.dram_tensor("xT", [F_IN, NSHP], dt.float32, kind="ExternalInput").ap()
    w1_in = nc.dram_tensor("w1ext", [F_IN, C + 2], dt.float32, kind="ExternalInput").ap()
    w2_in = nc.dram_tensor("w2ext", [C, C + 2], dt.float32, kind="ExternalInput").ap()
    b1_in = nc.dram_tensor("b1r", [1, C], dt.float32, kind="ExternalInput").ap()
    b2_in = nc.dram_tensor("b2r", [1, C], dt.float32, kind="ExternalInput").ap()
    fc1w_in = nc.dram_tensor("fc1w", [C, HID], dt.float32, kind="ExternalInput").ap()
    fc1b_in = nc.dram_tensor("fc1b", [1, HID], dt.float32, kind="ExternalInput").ap()
    fc2w_in = nc.dram_tensor("fc2w", [HID, NCLS], dt.float32, kind="ExternalInput").ap()
    fc2b_in = nc.dram_tensor("fc2b", [1, NCLS], dt.float32, kind="ExternalInput").ap()
    rcnt_in = nc.dram_tensor("rcnt", [G, 1], dt.float32, kind="ExternalInput").ap()
    g01_in = nc.dram_tensor("g01", [NSHP, 2], dt.float32, kind="ExternalInput").ap()
    iota_in = nc.dram_tensor("iota128", [P, P], dt.bfloat16, kind="ExternalInput").ap()
    idxlo_in = nc.dram_tensor("idxlo", [NSB, P, SBT * CLn * 8], dt.int16, kind="ExternalInput").ap()
    idxhi_in = nc.dram_tensor("idxhi", [NSB, P, SBT * CHn * 8], dt.int16, kind="ExternalInput").ap()
    dstloc_in = nc.dram_tensor("dstloc", [NSB, P, M], dt.float32, kind="ExternalInput").ap()
    adidx_in = nc.dram_tensor("adidx", [NSB, P, M * 8], dt.int16, kind="ExternalInput").ap()
    out_t = nc.dram_tensor("out", [G, NCLS], dt.float32, kind="ExternalOutput").ap()

    rg = [list(range(W))]

    with tile.TileContext(nc) as tc:
        with tc.tile_pool(name="consts", bufs=1) as consts, \
             tc.tile_pool(name="dram", bufs=1, space="DRAM") as dram:

            # ------------- constants -------------
            iota_sb = consts.tile([P, P], dt.bfloat16)
            nc.sync.dma_start(out=iota_sb[:], in_=iota_in[:])
            ident = consts.tile([P, P], dt.float32)
            make_identity(nc, ident[:])
            ones_col = consts.tile([P, 1], dt.bfloat16)
            nc.vector.memset(ones_col[:], 1.0)
            ones1 = consts.tile([1, P], dt.float32)
            nc.vector.memset(ones1[:], 1.0)

            w1_sb = []
            for k in range(3):
                kk = min(128, F_IN - k * 128)
                t_ = consts.tile([P, C + 2], dt.float32, name=f"w1sb{k}")
                nc.sync.dma_start(out=t_[:kk, :], in_=w1_in[k * 128:k * 128 + kk, :])
                w1_sb.append(t_)
            w2_sb = []
            for k in range(2):
                t_ = consts.tile([P, C + 2], dt.float32, name=f"w2sb{k}")
                nc.sync.dma_start(out=t_[:], in_=w2_in[k * 128:(k + 1) * 128, :])
                w2_sb.append(t_)
            fc1w_sb = []
            for k in range(2):
                t_ = consts.tile([P, HID], dt.float32, name=f"fc1wsb{k}")
                nc.sync.dma_start(out=t_[:], in_=fc1w_in[k * 128:(k + 1) * 128, :])
                fc1w_sb.append(t_)
            fc2w_sb = consts.tile([P, NCLS], dt.float32)
            nc.sync.dma_start(out=fc2w_sb[:], in_=fc2w_in[:])
            rcnt_sb = consts.tile([P, 2], dt.float32)
            nc.sync.dma_start(out=rcnt_sb[:, 0:1], in_=rcnt_in[0:128, :])
            nc.sync.dma_start(out=rcnt_sb[:, 1:2], in_=rcnt_in[128:256, :])

            # bias broadcasts via K=1 ones-matmul (psum pool scoped tightly)
            with tc.tile_pool(name="cpsum", bufs=1, space="PSUM") as cpsum:
                def bcast_row(src_ap, width, name):
                    row = consts.tile([1, width], dt.float32, name=f"{name}_row")
                    nc.sync.dma_start(out=row[:], in_=src_ap)
                    ps = cpsum.tile([P, width], dt.float32, space="PSUM",
                                    name=f"{name}_ps", tag="bps")
                    nc.tensor.matmul(ps[:], ones1[:, :], row[:], start=True, stop=True)
                    bb = consts.tile([P, width], dt.float32, name=f"{name}_bc")
                    nc.vector.tensor_copy(bb[:], ps[:])
                    return bb

                b1_bc = bcast_row(b1_in[:], C, "b1")
                b2_bc = bcast_row(b2_in[:], C, "b2")
                fc1b_bc = bcast_row(fc1b_in[:], HID, "fc1b")
                fc2b_bc = bcast_row(fc2b_in[:], NCLS, "fc2b")

            # resident transposed layer-1 output (lhsT for layer-2 node matmul)
            h1T = consts.tile([P, 2 * NSHP], dt.float32)

            # ------------- DRAM scratch -------------
            h_sh = [dram.tile([NSHP, HW_], dt.bfloat16, name=f"hsh{l}") for l in range(2)]
            ad_tab = [dram.tile([NSHP, 128], dt.bfloat16, name=f"adtab{l}") for l in range(2)]
            h_full = [dram.tile([NP, HW_], dt.bfloat16, addr_space="Shared", name=f"hfull{l}")
                      for l in range(2)]
            ppart = dram.tile([G, C], dt.float32)
            psum_g = dram.tile([G, C], dt.float32, addr_space="Shared")

            # ------------- helpers -------------
            def selu_into(pool, xsb, name, out_dtype=None):
                """returns tile = selu(xsb), 2 ACT + 4 DVE ops."""
                shp = list(xsb.shape)
                r2 = pool.tile(shp, dt.float32, name=f"{name}_r2")
                nc.scalar.activation(r2[:], xsb, act.Relu, scale=-1.0)
                en = pool.tile(shp, dt.float32, name=f"{name}_en")
                nc.scalar.activation(en[:], r2[:], act.Exp, scale=-1.0)
                t1 = pool.tile(shp, dt.float32, name=f"{name}_t1")
                nc.vector.tensor_scalar(t1[:], en[:], LA, -LA, op.mult, op.add)
                t2 = pool.tile(shp, dt.float32, name=f"{name}_t2")
                nc.vector.tensor_tensor(out=t2[:], in0=xsb, in1=r2[:], op=op.add)
                t3 = pool.tile(shp, dt.float32, name=f"{name}_t3")
                nc.vector.tensor_scalar(t3[:], t2[:], LAM, None, op.mult)
                res = pool.tile(shp, out_dtype or dt.float32, name=f"{name}_res")
                nc.vector.tensor_tensor(out=res[:], in0=t1[:], in1=t3[:], op=op.add)
                return res

            # ------------- node phase -------------
            def node_phase(layer):
                w_sb = w1_sb if layer == 0 else w2_sb
                nk = 3 if layer == 0 else 2
                with tc.tile_pool(name=f"np{layer}", bufs=3) as npool, \
                     tc.tile_pool(name=f"npp{layer}", bufs=2, space="PSUM") as nppsum:
                    for t in range(TPC):
                        pn = nppsum.tile([P, C + 2], dt.float32, space="PSUM", name="pn")
                        for k in range(nk):
                            kk = min(128, (F_IN if layer == 0 else C) - k * 128)
                            if layer == 0:
                                xt = npool.tile([P, P], dt.float32, name="xt")
                                nc.sync.dma_start(
                                    out=xt[:kk, :],
                                    in_=xT_in[k * 128:k * 128 + kk, t * 128:(t + 1) * 128])
                                lhsT = xt[:kk, :]
                            else:
                                lhsT = h1T[:, k * NSHP + t * 128:k * NSHP + (t + 1) * 128]
                            nc.tensor.matmul(pn[:], lhsT, w_sb[k][:kk, :],
                                             start=(k == 0), stop=(k == nk - 1))
                        hbf = npool.tile([P, C], dt.bfloat16, name="hbf")
                        nc.vector.tensor_copy(hbf[:], pn[:, 0:C])
                        acol = npool.tile([P, 2], dt.float32, name="acol")
                        nc.vector.tensor_copy(acol[:], pn[:, C:C + 2])
                        adbf = npool.tile([P, 1], dt.bfloat16, name="adbf")
                        nc.vector.tensor_copy(adbf[:], pn[:, C + 1:C + 2])
                        nc.sync.dma_start(out=h_sh[layer][t * 128:(t + 1) * 128, 0:C], in_=hbf[:])
                        nc.sync.dma_start(
                            out=h_sh[layer][t * 128:(t + 1) * 128, C:C + 2].bitcast(dt.float32),
                            in_=acol[:, 0:1])
                        nc.sync.dma_start(out=ad_tab[layer][t * 128:(t + 1) * 128, 0:1], in_=adbf[:])

            # ------------- edge phase -------------
            def edge_phase(layer, ppool0=None, ppool1=None):
                CLg = SBT * CLn
                with tc.tile_pool(name=f"gb{layer}", bufs=2) as gpool, \
                     tc.tile_pool(name=f"st{layer}", bufs=2) as stpool, \
                     tc.tile_pool(name=f"s{layer}", bufs=6) as spool, \
                     tc.tile_pool(name=f"ep{layer}", bufs=2) as epool, \
                     tc.tile_pool(name=f"pm{layer}", bufs=2, space="PSUM") as pmpool, \
                     tc.tile_pool(name=f"pd{layer}", bufs=2, space="PSUM") as pdpool, \
                     tc.tile_pool(name=f"pt{layer}", bufs=2, space="PSUM") as ptpool:
                    if "no_edge" in DEBUG_FLAGS:
                        nc.vector.memset(h1T[:], 0.1)
                        if layer == 1:
                            Sz = spool.tile([P, P], dt.bfloat16, name="Sz")
                            nc.vector.memset(Sz[:], 0.0)
                            hz = epool.tile([P, C], dt.bfloat16, name="hz")
                            nc.vector.memset(hz[:], 0.0)
                            nc.tensor.matmul(ppool0[:], Sz[:], hz[:], start=True, stop=True)
                            nc.tensor.matmul(ppool1[:], Sz[:], hz[:], start=True, stop=True)
                        return
                    for sb in range(NSB):
                        ilo = stpool.tile([P, SBT * CLn * 8], dt.int16, name="ilo")
                        nc.sync.dma_start(out=ilo[:], in_=idxlo_in[sb])
                        ihi = stpool.tile([P, SBT * CHn * 8], dt.int16, name="ihi")
                        nc.sync.dma_start(out=ihi[:], in_=idxhi_in[sb])
                        dl_sb = stpool.tile([P, M], dt.float32, name="dl_sb")
                        nc.sync.dma_start(out=dl_sb[:], in_=dstloc_in[sb])
                        adi = stpool.tile([P, M * 8], dt.int16, name="adi")
                        nc.sync.dma_start(out=adi[:], in_=adidx_in[sb])

                        gbuf = gpool.tile([P, M, HW_], dt.bfloat16, name="gbuf")
                        adg = gpool.tile([P, M, 128], dt.bfloat16, name="adg")
                        if "no_gather" in DEBUG_FLAGS:
                            nc.vector.memset(gbuf[:].rearrange("p a b -> p (a b)"), 0.25)
                            nc.vector.memset(adg[:].rearrange("p a b -> p (a b)"), 0.1)
                        else:
                            nc.gpsimd.dma_gather(
                                gbuf[:, 0:CLg, :], h_full[layer][:], ilo[:],
                                num_idxs=CLg * P, num_idxs_reg=CLg * P, elem_size=HW_,
                                single_packet=False, queue_num=0)
                            nc.gpsimd.dma_gather(
                                gbuf[:, CLg:M, :], h_full[layer][SPLIT:, :], ihi[:],
                                num_idxs=SBT * CHn * P, num_idxs_reg=SBT * CHn * P,
                                elem_size=HW_, single_packet=False, queue_num=1)
                            nc.gpsimd.dma_gather(
                                adg[:], ad_tab[layer][:], adi[:],
                                num_idxs=M * P, num_idxs_reg=M * P, elem_size=128,
                                single_packet=False, queue_num=2)

                        as_v = gbuf[:, :, C:C + 2].bitcast(dt.float32).rearrange(
                            "p m o -> p (m o)")
                        ad_v = adg[:, :, 0]
                        e_t = stpool.tile([P, M], dt.float32, name="e_t")
                        nc.vector.tensor_tensor(out=e_t[:], in0=as_v, in1=ad_v, op=op.add)
                        l_t = stpool.tile([P, M], dt.float32, name="l_t")
                        nc.scalar.activation(l_t[:], e_t[:], act.Prelu, alpha=0.2)
                        ex_t = stpool.tile([P, M], dt.float32, name="ex_t")
                        nc.scalar.activation(ex_t[:], l_t[:], act.Exp)

                        for j in range(SBT):
                            t = sb * SBT + j
                            chunks = ([j * CLn + k for k in range(CLn)] +
                                      [CLg + j * CHn + k for k in range(CHn)])
                            pm = pmpool.tile([P, C], dt.float32, space="PSUM", name="pm")
                            pd = pdpool.tile([P, 8], dt.float32, space="PSUM", name="pd")
                            for ci, n in enumerate(chunks):
                                S = spool.tile([P, P], dt.bfloat16, name="S")
                                nc.vector.tensor_scalar(
                                    S[:], iota_sb[:], dl_sb[:, n:n + 1], ex_t[:, n:n + 1],
                                    op.is_equal, op.mult)
                                nc.tensor.matmul(pm[:], S[:], gbuf[:, n, 0:C],
                                                 start=(ci == 0), stop=(ci == CT - 1))
                                nc.tensor.matmul(pd[:, 0:1], S[:], ones_col[:],
                                                 start=(ci == 0), stop=(ci == CT - 1))
                            if t >= TPC:
                                continue
                            # epilogue
                            dcl = epool.tile([P, 1], dt.float32, name="dcl")
                            nc.vector.tensor_scalar(dcl[:], pd[:, 0:1], 1e-30, None, op.max)
                            rec = epool.tile([P, 1], dt.float32, name="rec")
                            nc.vector.reciprocal(rec[:], dcl[:])
                            osb = epool.tile([P, C], dt.float32, name="osb")
                            nc.vector.tensor_scalar(osb[:], pm[:], rec[:], None, op.mult)
                            xsb = epool.tile([P, C], dt.float32, name="xsb")
                            nc.vector.tensor_tensor(
                                out=xsb[:], in0=osb[:],
                                in1=(b1_bc if layer == 0 else b2_bc)[:], op=op.add)
                            hout = selu_into(epool, xsb[:], "selu_e")
                            if layer == 0:
                                for k in range(2):
                                    ptr = ptpool.tile([P, P], dt.float32, space="PSUM", name="ptr")
                                    nc.tensor.transpose(
                                        ptr[:], hout[:, k * 128:(k + 1) * 128], ident[:])
                                    nc.vector.tensor_copy(
                                        h1T[:, k * NSHP + t * 128:k * NSHP + (t + 1) * 128],
                                        ptr[:])
                            else:
                                hb2 = epool.tile([P, C], dt.bfloat16, name="hb2")
                                nc.vector.tensor_copy(hb2[:], hout[:])
                                g01 = epool.tile([P, 2], dt.float32, name="g01")
                                nc.sync.dma_start(
                                    out=g01[:], in_=g01_in[t * 128:(t + 1) * 128, :])
                                for b_ in range(2):
                                    S0 = spool.tile([P, P], dt.bfloat16, name="S0")
                                    nc.vector.tensor_scalar(
                                        S0[:], iota_sb[:], g01[:, b_:b_ + 1], None, op.is_equal)
                                    nc.tensor.matmul(
                                        (ppool0 if b_ == 0 else ppool1)[:], S0[:], hb2[:],
                                        start=(t == 0), stop=(t == TPC - 1))

            # ================= pipeline =================
            node_phase(0)
            nc.gpsimd.collective_compute(
                "AllGather", mybir.AluOpType.bypass, replica_groups=rg,
                ins=[h_sh[0][:]], outs=[h_full[0][:]])
            edge_phase(0)
            node_phase(1)
            nc.gpsimd.collective_compute(
                "AllGather", mybir.AluOpType.bypass, replica_groups=rg,
                ins=[h_sh[1][:]], outs=[h_full[1][:]])

            with tc.tile_pool(name="poolp", bufs=1, space="PSUM") as plpool:
                ppool0 = plpool.tile([P, C], dt.float32, space="PSUM", name="ppool0")
                ppool1 = plpool.tile([P, C], dt.float32, space="PSUM", name="ppool1")
                edge_phase(1, ppool0, ppool1)

                # ------------- head -------------
                with tc.tile_pool(name="head", bufs=1) as hp, \
                     tc.tile_pool(name="hps", bufs=3, space="PSUM") as hps:
                    pool_sb = hp.tile([P, 2 * C], dt.float32)
                    nc.vector.tensor_copy(pool_sb[:, 0:C], ppool0[:])
                    nc.vector.tensor_copy(pool_sb[:, C:2 * C], ppool1[:])
                    nc.sync.dma_start(out=ppart[0:128, :], in_=pool_sb[:, 0:C])
                    nc.sync.dma_start(out=ppart[128:256, :], in_=pool_sb[:, C:2 * C])
                    nc.gpsimd.collective_compute(
                        "AllReduce", mybir.AluOpType.add, replica_groups=rg,
                        ins=[ppart[:]], outs=[psum_g[:]])
                    for b_ in range(2):
                        gs = hp.tile([P, C], dt.float32, name=f"gs{b_}")
                        nc.sync.dma_start(out=gs[:], in_=psum_g[b_ * 128:(b_ + 1) * 128, :])
                        gm = hp.tile([P, C], dt.float32, name=f"gm{b_}")
                        nc.vector.tensor_scalar(gm[:], gs[:], rcnt_sb[:, b_:b_ + 1], None, op.mult)
                        gsel = selu_into(hp, gm[:], f"selu_g{b_}")
                        # fc1: transpose gsel -> 2 chunks, matmul with fc1w
                        pf1 = hps.tile([P, HID], dt.float32, space="PSUM", name=f"pf1_{b_}", tag="hpsum")
                        gTs = []
                        for k in range(2):
                            ptr = hps.tile([P, P], dt.float32, space="PSUM", name=f"gT_ps{b_}{k}", tag="hpsum")
                            nc.tensor.transpose(ptr[:], gsel[:, k * 128:(k + 1) * 128], ident[:])
                            gT = hp.tile([P, P], dt.float32, name=f"gT{b_}{k}")
                            nc.vector.tensor_copy(gT[:], ptr[:])
                            gTs.append(gT)
                        for k in range(2):
                            nc.tensor.matmul(pf1[:], gTs[k][:], fc1w_sb[k][:],
                                             start=(k == 0), stop=(k == 1))
                        x1 = hp.tile([P, HID], dt.float32, name=f"x1_{b_}")
                        nc.vector.tensor_tensor(out=x1[:], in0=pf1[:], in1=fc1b_bc[:], op=op.add)
                        g1 = selu_into(hp, x1[:], f"selu_f{b_}")
                        # fc2
                        ptr2 = hps.tile([P, P], dt.float32, space="PSUM", name=f"g1T_ps{b_}", tag="hpsum")
                        nc.tensor.transpose(ptr2[:], g1[:], ident[:])
                        g1T = hp.tile([P, P], dt.float32, name=f"g1T{b_}")
                        nc.vector.tensor_copy(g1T[:], ptr2[:])
                        pf2 = hps.tile([P, 8], dt.float32, space="PSUM", name=f"pf2_{b_}", tag="hpsum")
                        nc.tensor.matmul(pf2[:, 0:NCLS], g1T[:], fc2w_sb[:], start=True, stop=True)
                        x2 = hp.tile([P, NCLS], dt.float32, name=f"x2_{b_}")
                        nc.vector.tensor_tensor(out=x2[:], in0=pf2[:, 0:NCLS], in1=fc2b_bc[:], op=op.add)
                        # log_softmax
                        mx = hp.tile([P, 1], dt.float32, name=f"mx{b_}")
                        nc.vector.tensor_reduce(mx[:], x2[:], axis=mybir.AxisListType.X, op=op.max)
                        zc = hp.tile([P, NCLS], dt.float32, name=f"zc{b_}")
                        nc.vector.tensor_scalar(zc[:], x2[:], mx[:], None, op.subtract)
                        ee = hp.tile([P, NCLS], dt.float32, name=f"ee{b_}")
                        nc.scalar.activation(ee[:], zc[:], act.Exp)
                        ssum = hp.tile([P, 1], dt.float32, name=f"ss{b_}")
                        nc.vector.tensor_reduce(ssum[:], ee[:], axis=mybir.AxisListType.X, op=op.add)
                        lls = hp.tile([P, 1], dt.float32, name=f"ll{b_}")
                        nc.scalar.activation(lls[:], ssum[:], act.Ln)
                        oo = hp.tile([P, NCLS], dt.float32, name=f"oo{b_}")
                        nc.vector.tensor_scalar(oo[:], zc[:], lls[:], None, op.subtract)
                        nc.sync.dma_start(out=out_t[b_ * 128:(b_ + 1) * 128, :], in_=oo[:])

    nc.compile()
    return nc


def kernel(**inputs):
    import concourse.bass  # noqa: F401  (path setup)
    from concourse.bass_utils import run_bass_kernel_spmd

    edge_index = np.asarray(inputs["edge_index"], np.int64)
    ep = _edge_prep(edge_index)
    key = (ep["CL"], ep["CH"])
    if key not in _CACHE:
        _CACHE[key] = _build(*key)
    nc = _CACHE[key]
    in_maps = _make_in_maps(inputs, ep)
    res = run_bass_kernel_spmd(nc, in_maps, core_ids=list(range(W)))
    return np.asarray(res.results[0]["out"], np.float32)


def _make_in_maps(inputs, ep):
    x = np.asarray(inputs["x"], np.float32)
    batch = np.asarray(inputs["batch"], np.int64)
    W1 = np.asarray(inputs["W1"], np.float32)
    W2 = np.asarray(inputs["W2"], np.float32)

    def ext(Wm, al, ar):
        Wm64 = Wm.astype(np.float64)
        return np.concatenate(
            [Wm, (Wm64 @ np.asarray(al, np.float64))[:, None].astype(np.float32),
             (Wm64 @ np.asarray(ar, np.float64))[:, None].astype(np.float32)], axis=1)

    w1ext = ext(W1, inputs["att_l1"], inputs["att_r1"])
    w2ext = ext(W2, inputs["att_l2"], inputs["att_r2"])
    cnt = np.bincount(batch, minlength=G).astype(np.float32)
    rcnt = (1.0 / np.maximum(cnt, 1.0)).reshape(G, 1)
    iota = np.broadcast_to(np.arange(P, dtype=np.float32)[None, :], (P, P)).astype(BF16)

    in_maps = []
    for c in range(W):
        xs = np.zeros((NSHP, F_IN), np.float32)
        xs[:NSH] = x[c * NSH:(c + 1) * NSH]
        g01 = np.full((NSHP, 2), 1e9, np.float32)
        bsh = batch[c * NSH:(c + 1) * NSH].astype(np.float32)
        g01[:NSH, 0] = bsh
        g01[:NSH, 1] = bsh - 128.0
        in_maps.append({
            "xT": np.ascontiguousarray(xs.T),
            "w1ext": w1ext, "w2ext": w2ext,
            "b1r": np.asarray(inputs["b1"], np.float32).reshape(1, C),
            "b2r": np.asarray(inputs["b2"], np.float32).reshape(1, C),
            "fc1w": np.asarray(inputs["fc1_W"], np.float32),
            "fc1b": np.asarray(inputs["fc1_b"], np.float32).reshape(1, HID),
            "fc2w": np.asarray(inputs["fc2_W"], np.float32),
            "fc2b": np.asarray(inputs["fc2_b"], np.float32).reshape(1, NCLS),
            "rcnt": rcnt, "g01": g01, "iota128": iota,
            "idxlo": ep["idxlo"][c], "idxhi": ep["idxhi"][c],
            "dstloc": ep["dstloc"][c], "adidx": ep["adidx"][c],
        })
    return in_maps


def benchmark(inputs, iters=12):
    """Repeated sharded execution with device-resident inputs; returns min ns."""
    import time
    import jax
    from jax.sharding import Mesh, PartitionSpec, NamedSharding
    from jax.experimental.shard_map import shard_map
    from concourse import bass2jax
    import concourse.mybir as mybir
    bass2jax.install_neuronx_cc_hook()

    edge_index = np.asarray(inputs["edge_index"], np.int64)
    ep = _edge_prep(edge_index)
    key = (ep["CL"], ep["CH"])
    if key not in _CACHE:
        _CACHE[key] = _build(*key)
    nc = _CACHE[key]
    in_maps = _make_in_maps(inputs, ep)

    part_name = nc.partition_id_tensor.name if nc.partition_id_tensor else None
    in_names, out_names, out_avals, zero_outs = [], [], [], []
    for alloc in nc.m.functions[0].allocations:
        if not isinstance(alloc, mybir.MemoryLocationSet):
            continue
        name = alloc.memorylocations[0].name
        if alloc.kind == "ExternalInput":
            if name != part_name:
                in_names.append(name)
        elif alloc.kind == "ExternalOutput":
            out_names.append(name)
            shape = tuple(alloc.tensor_shape)
            dtype = mybir.dt.np(alloc.dtype)
            out_avals.append(jax.core.ShapedArray(shape, dtype))
            zero_outs.append(np.zeros(shape, dtype))
    n_params = len(in_names)
    all_in = in_names + out_names
    if part_name is not None:
        all_in = all_in + [part_name]

    def _body(*args):
        operands = list(args)
        if part_name is not None:
            operands.append(bass2jax.partition_id_tensor())
        outs = bass2jax._bass_exec_p.bind(
            *operands, out_avals=tuple(out_avals), in_names=tuple(all_in),
            out_names=tuple(out_names), lowering_input_output_aliases=(),
            sim_require_finite=True, sim_require_nnan=True, nc=nc)
        return tuple(outs)

    devices = jax.devices()[:W]
    mesh = Mesh(np.asarray(devices), ("core",))
    nin = n_params + len(out_names)
    f1 = jax.jit(shard_map(_body, mesh=mesh, in_specs=(PartitionSpec("core"),) * nin,
                           out_specs=(PartitionSpec("core"),) * len(out_names),
                           check_rep=False), keep_unused=True)
    concat_in = [np.concatenate([in_maps[c][nm] for c in range(W)], axis=0)
                 for nm in in_names]
    concat_zero = [np.zeros((W * z.shape[0], *z.shape[1:]), z.dtype) for z in zero_outs]
    sh = NamedSharding(mesh, PartitionSpec("core"))
    dev_args = [jax.device_put(a, sh) for a in concat_in + concat_zero]
    jax.block_until_ready(f1(*dev_args))

    # async pipelined submissions: device executions serialize per-core while
    # dispatch overlaps, so (t_N - t_1)/(N-1) isolates per-execution time.
    def run_n(n):
        ts = []
        for _ in range(iters):
            t0 = time.perf_counter()
            r = None
            for _ in range(n):
                r = f1(*dev_args)
            jax.block_until_ready(r)
            ts.append(time.perf_counter() - t0)
        ts.sort()
        return ts[0]

    NCH = 16
    t1 = run_n(1)
    tN = run_n(NCH)
    per_exec = (tN - t1) / (NCH - 1)
    print(f"  t1 min={t1*1e3:.3f}ms | t{NCH} min={tN*1e3:.3f}ms -> per-exec {per_exec*1e6:.1f}us")
    return per_exec * 1e9


if __name__ == "__main__":
    import time
    ei = np.load("/tmp/edge_index.npy")
    t0 = time.time()
    ep = _edge_prep(ei)
    print("edge prep:", round(time.time() - t0, 1), "s; CL/CH:", ep["CL"], ep["CH"])


# revision 19
# speedup vs baseline: 1.7187x; 1.7187x over previous
"""Trainium2 Bass kernel for nn_GCNFN (2-layer GAT + mean-pool + MLP head).

Strategy (8 NeuronCores, graph-parallel by destination node):
  * Nodes are sharded 6250/core (padded 6272). Edges (with self-loops) are
    sorted by dst and assigned to the core owning the dst node.
  * Per layer: node phase computes h_ext = x @ [W | W@att_l | W@att_r] for the
    local shard; h (bf16) and a_src (f32) are AllGathered; a_dst stays local.
  * Edge phase per dst-tile of 128 nodes: h[src] rows are fetched with
    dma_gather (bf16, 512B rows; src ids split into <32768 / >=32768 buckets
    for int16 indices), per-edge a_src/a_dst scalars with 4-byte indirect
    DMAs.  exp(leaky_relu(a_src+a_dst)) is computed compactly, expanded into a
    one-hot-times-exp matrix S^T [128 edges, 128 dst] with a single fused
    tensor_scalar op per 128-edge chunk, and the segment softmax-weighted sum
    becomes PE matmuls accumulated in PSUM (plus a ones-column matmul for the
    denominators).
  * Mean-pool partial sums are computed with host-known one-hot matmuls and
    AllReduced; the tiny MLP head + log_softmax run redundantly on every core.
"""

import sys

sys.path.insert(0, "/opt/trn_rl_repo")

import numpy as np
import ml_dtypes

BF16 = ml_dtypes.bfloat16

# ---- problem constants (hardcoded) ----
N = 50000
E0 = 800000
G = 256
F_IN = 310
C = 256
HID = 128
NCLS = 2
W = 8            # cores
NSH = 6250       # nodes per core
NSHP = 6272      # padded (49*128)
NP = NSHP * W    # 50176 padded global rows
P = 128
TPC = 49         # dst tiles per core
SBT = 2          # tiles per superblock
NSB = 25         # superblocks (50 tile slots, 1 dummy)
HW_ = 384        # gathered row width (bf16 units): 256 h + 2 a_src(f32) + pad
TPC_PAD = NSB * SBT
SPLIT = 32768    # int16 gather index limit

LAM = 1.0507009873554805
ALPHA_SELU = 1.6732632423543772
LA = LAM * ALPHA_SELU

_CACHE = {}
DEBUG_FLAGS = set()  # {"no_gather","no_indirect","no_edge","no_coll"}


def _edge_prep(edge_index):
    """Host-side edge preprocessing -> per-core index streams.

    Nodes are permuted within each shard (degree-balanced deal across the 49
    dst tiles) so per-tile edge counts equalize, minimizing chunk padding.
    """
    src = np.concatenate([edge_index[0], np.arange(N)]).astype(np.int64)
    dst = np.concatenate([edge_index[1], np.arange(N)]).astype(np.int64)

    # in-degree per node -> per-shard permutation old_local -> new_local
    deg = np.bincount(dst, minlength=N)
    newloc = np.zeros(N, np.int64)         # per-node new LOCAL id within shard
    for c in range(W):
        d = deg[c * NSH:(c + 1) * NSH]
        rank = np.argsort(np.argsort(-d, kind="stable"), kind="stable")
        newloc[c * NSH:(c + 1) * NSH] = (rank % TPC) * P + rank // TPC

    src_new = (src // NSH) * NSHP + newloc[src]     # padded global, permuted
    dst_core = dst // NSH
    dst_loc = newloc[dst]                            # new local id

    order = np.argsort(dst_core * NSHP + dst_loc, kind="stable")
    srcp = src_new[order]
    core_of = dst_core[order]
    dl_sh = dst_loc[order]
    tile_of = dl_sh // P
    dloc = dl_sh % P
    low = srcp < SPLIT

    key = core_of * TPC + tile_of
    nlow = np.bincount(key[low], minlength=W * TPC).reshape(W, TPC)
    nhigh = np.bincount(key[~low], minlength=W * TPC).reshape(W, TPC)
    CLn = int(np.ceil(nlow.max() / P))
    CHn = int(np.ceil(nhigh.max() / P))
    CT = CLn + CHn
    M = SBT * CT

    idxlo = np.zeros((W, NSB, P, SBT * CLn * 8), np.int16)
    idxhi = np.zeros((W, NSB, P, SBT * CHn * 8), np.int16)
    adidx = np.zeros((W, NSB, P, M * 8), np.int16)
    dstloc = np.full((W, NSB, P, M), 255.0, np.float32)

    for c in range(W):
        msk_c = core_of == c
        for t in range(TPC):
            sb, j = divmod(t, SBT)
            msk_t = msk_c & (tile_of == t)
            for is_low, CB, chunk0, idxarr, vals_off in (
                (True, CLn, j * CLn, idxlo, 0),
                (False, CHn, SBT * CLn + j * CHn, idxhi, SPLIT),
            ):
                m = msk_t & (low if is_low else ~low)
                vals = srcp[m] - vals_off          # gather indices (int16 range)
                dl = dloc[m]
                asv = srcp[m]                      # a_src gather ids (global padded)
                adv = dl_sh[m]                     # a_dst gather ids (local)
                n = len(vals)
                assert n <= CB * P
                q = np.arange(n)
                # gather index list (wrapped [i%16, i//16], replicated x8)
                Lpos = (j if is_low else j) * CB * P + q
                rows = (Lpos % 16)
                cols = Lpos // 16
                for g in range(8):
                    idxarr[c, sb, g * 16 + rows, cols] = vals.astype(np.int16)
                # gather padding -> index 0 (valid dummy row)
                # (array already zero-initialized)
                pp = q % P
                ch = chunk0 + q // P
                dstloc[c, sb, pp, ch] = dl.astype(np.float32)
                # a_dst gather list position = ch*128 + pp (call-chunk order)
                Qpos = ch * P + pp
                for g in range(8):
                    adidx[c, sb, g * 16 + Qpos % 16, Qpos // 16] = adv.astype(np.int16)

    return dict(CL=CLn, CH=CHn, CT=CT, M=M, idxlo=idxlo, idxhi=idxhi,
                dstloc=dstloc, adidx=adidx, newloc=newloc)


def _build(CLn, CHn):
    import concourse.bass as bass
    import concourse.bacc as bacc
    import concourse.tile as tile
    import concourse.mybir as mybir
    from concourse.masks import make_identity

    dt = mybir.dt
    op = mybir.AluOpType
    act = mybir.ActivationFunctionType

    CT = CLn + CHn
    M = SBT * CT

    nc = bacc.Bacc("TRN2", target_bir_lowering=False, debug=False,
                   enable_asserts=False, num_devices=W, num_swdge_queues=4)

    # ---------------- I/O ----------------
    xT_in = nc.dram_tensor("xT", [F_IN, NSHP], dt.float32, kind="ExternalInput").ap()
    w1_in = nc.dram_tensor("w1ext", [F_IN, C + 2], dt.float32, kind="ExternalInput").ap()
    w2_in = nc.dram_tensor("w2ext", [C, C + 2], dt.float32, kind="ExternalInput").ap()
    b1_in = nc.dram_tensor("b1r", [1, C], dt.float32, kind="ExternalInput").ap()
    b2_in = nc.dram_tensor("b2r", [1, C], dt.float32, kind="ExternalInput").ap()
    fc1w_in = nc.dram_tensor("fc1w", [C, HID], dt.float32, kind="ExternalInput").ap()
    fc1b_in = nc.dram_tensor("fc1b", [1, HID], dt.float32, kind="ExternalInput").ap()
    fc2w_in = nc.dram_tensor("fc2w", [HID, NCLS], dt.float32, kind="ExternalInput").ap()
    fc2b_in = nc.dram_tensor("fc2b", [1, NCLS], dt.float32, kind="ExternalInput").ap()
    rcnt_in = nc.dram_tensor("rcnt", [G, 1], dt.float32, kind="ExternalInput").ap()
    g01_in = nc.dram_tensor("g01", [NSHP, 2], dt.float32, kind="ExternalInput").ap()
    iota_in = nc.dram_tensor("iota128", [P, P], dt.bfloat16, kind="ExternalInput").ap()
    idxlo_in = nc.dram_tensor("idxlo", [NSB, P, SBT * CLn * 8], dt.int16, kind="ExternalInput").ap()
    idxhi_in = nc.dram_tensor("idxhi", [NSB, P, SBT * CHn * 8], dt.int16, kind="ExternalInput").ap()
    dstloc_in = nc.dram_tensor("dstloc", [NSB, P, M], dt.float32, kind="ExternalInput").ap()
    adidx_in = nc.dram_tensor("adidx", [NSB, P, M * 8], dt.int16, kind="ExternalInput").ap()
    out_t = nc.dram_tensor("out", [G, NCLS], dt.float32, kind="ExternalOutput").ap()

    rg = [list(range(W))]

    with tile.TileContext(nc) as tc:
        with tc.tile_pool(name="consts", bufs=1) as consts, \
             tc.tile_pool(name="dram", bufs=1, space="DRAM") as dram:

            # ------------- constants -------------
            iota_sb = consts.tile([P, P], dt.bfloat16)
            nc.sync.dma_start(out=iota_sb[:], in_=iota_in[:])
            ident = consts.tile([P, P], dt.float32)
            make_identity(nc, ident[:])
            ones_col = consts.tile([P, 1], dt.bfloat16)
            nc.vector.memset(ones_col[:], 1.0)
            ones1 = consts.tile([1, P], dt.float32)
            nc.vector.memset(ones1[:], 1.0)

            w1_sb = []
            for k in range(3):
                kk = min(128, F_IN - k * 128)
                t_ = consts.tile([P, C + 2], dt.float32, name=f"w1sb{k}")
                nc.sync.dma_start(out=t_[:kk, :], in_=w1_in[k * 128:k * 128 + kk, :])
                w1_sb.append(t_)
            w2_sb = []
            for k in range(2):
                t_ = consts.tile([P, C + 2], dt.float32, name=f"w2sb{k}")
                nc.sync.dma_start(out=t_[:], in_=w2_in[k * 128:(k + 1) * 128, :])
                w2_sb.append(t_)
            fc1w_sb = []
            for k in range(2):
                t_ = consts.tile([P, HID], dt.float32, name=f"fc1wsb{k}")
                nc.sync.dma_start(out=t_[:], in_=fc1w_in[k * 128:(k + 1) * 128, :])
                fc1w_sb.append(t_)
            fc2w_sb = consts.tile([P, NCLS], dt.float32)
            nc.sync.dma_start(out=fc2w_sb[:], in_=fc2w_in[:])
            rcnt_sb = consts.tile([P, 2], dt.float32)
            nc.sync.dma_start(out=rcnt_sb[:, 0:1], in_=rcnt_in[0:128, :])
            nc.sync.dma_start(out=rcnt_sb[:, 1:2], in_=rcnt_in[128:256, :])

            # bias broadcasts via K=1 ones-matmul (psum pool scoped tightly)
            with tc.tile_pool(name="cpsum", bufs=1, space="PSUM") as cpsum:
                def bcast_row(src_ap, width, name):
                    row = consts.tile([1, width], dt.float32, name=f"{name}_row")
                    nc.sync.dma_start(out=row[:], in_=src_ap)
                    ps = cpsum.tile([P, width], dt.float32, space="PSUM",
                                    name=f"{name}_ps", tag="bps")
                    nc.tensor.matmul(ps[:], ones1[:, :], row[:], start=True, stop=True)
                    bb = consts.tile([P, width], dt.float32, name=f"{name}_bc")
                    nc.vector.tensor_copy(bb[:], ps[:])
                    return bb

                b1_bc = bcast_row(b1_in[:], C, "b1")
                b2_bc = bcast_row(b2_in[:], C, "b2")
                fc1b_bc = bcast_row(fc1b_in[:], HID, "fc1b")
                fc2b_bc = bcast_row(fc2b_in[:], NCLS, "fc2b")

            # resident transposed layer-1 output (lhsT for layer-2 node matmul)
            h1T = consts.tile([P, 2 * NSHP], dt.float32)

            # ------------- DRAM scratch -------------
            h_sh = [dram.tile([NSHP, HW_], dt.bfloat16, name=f"hsh{l}") for l in range(2)]
            ad_tab = [dram.tile([NSHP, 128], dt.bfloat16, name=f"adtab{l}") for l in range(2)]
            h_full = [dram.tile([NP, HW_], dt.bfloat16, addr_space="Shared", name=f"hfull{l}")
                      for l in range(2)]
            ppart = dram.tile([G, C], dt.float32)
            psum_g = dram.tile([G, C], dt.float32, addr_space="Shared")

            # ------------- helpers -------------
            def selu_into(pool, xsb, name, out_dtype=None):
                """returns tile = selu(xsb), 2 ACT + 4 DVE ops."""
                shp = list(xsb.shape)
                r2 = pool.tile(shp, dt.float32, name=f"{name}_r2")
                nc.scalar.activation(r2[:], xsb, act.Relu, scale=-1.0)
                en = pool.tile(shp, dt.float32, name=f"{name}_en")
                nc.scalar.activation(en[:], r2[:], act.Exp, scale=-1.0)
                t1 = pool.tile(shp, dt.float32, name=f"{name}_t1")
                nc.vector.tensor_scalar(t1[:], en[:], LA, -LA, op.mult, op.add)
                t2 = pool.tile(shp, dt.float32, name=f"{name}_t2")
                nc.vector.tensor_tensor(out=t2[:], in0=xsb, in1=r2[:], op=op.add)
                t3 = pool.tile(shp, dt.float32, name=f"{name}_t3")
                nc.vector.tensor_scalar(t3[:], t2[:], LAM, None, op.mult)
                res = pool.tile(shp, out_dtype or dt.float32, name=f"{name}_res")
                nc.vector.tensor_tensor(out=res[:], in0=t1[:], in1=t3[:], op=op.add)
                return res

            # ------------- node phase -------------
            def node_phase(layer):
                w_sb = w1_sb if layer == 0 else w2_sb
                nk = 3 if layer == 0 else 2
                with tc.tile_pool(name=f"np{layer}", bufs=3) as npool, \
                     tc.tile_pool(name=f"npp{layer}", bufs=2, space="PSUM") as nppsum:
                    for t in range(TPC):
                        pn = nppsum.tile([P, C + 2], dt.float32, space="PSUM", name="pn")
                        for k in range(nk):
                            kk = min(128, (F_IN if layer == 0 else C) - k * 128)
                            if layer == 0:
                                xt = npool.tile([P, P], dt.float32, name="xt")
                                nc.sync.dma_start(
                                    out=xt[:kk, :],
                                    in_=xT_in[k * 128:k * 128 + kk, t * 128:(t + 1) * 128])
                                lhsT = xt[:kk, :]
                            else:
                                lhsT = h1T[:, k * NSHP + t * 128:k * NSHP + (t + 1) * 128]
                            nc.tensor.matmul(pn[:], lhsT, w_sb[k][:kk, :],
                                             start=(k == 0), stop=(k == nk - 1))
                        hbf = npool.tile([P, C], dt.bfloat16, name="hbf")
                        nc.vector.tensor_copy(hbf[:], pn[:, 0:C])
                        acol = npool.tile([P, 2], dt.float32, name="acol")
                        nc.vector.tensor_copy(acol[:], pn[:, C:C + 2])
                        adbf = npool.tile([P, 1], dt.bfloat16, name="adbf")
                        nc.vector.tensor_copy(adbf[:], pn[:, C + 1:C + 2])
                        nc.sync.dma_start(out=h_sh[layer][t * 128:(t + 1) * 128, 0:C], in_=hbf[:])
                        nc.sync.dma_start(
                            out=h_sh[layer][t * 128:(t + 1) * 128, C:C + 2].bitcast(dt.float32),
                            in_=acol[:, 0:1])
                        nc.sync.dma_start(out=ad_tab[layer][t * 128:(t + 1) * 128, 0:1], in_=adbf[:])

            # ------------- edge phase -------------
            def edge_phase(layer, ppool0=None, ppool1=None):
                CLg = SBT * CLn
                with tc.tile_pool(name=f"gb{layer}", bufs=2) as gpool, \
                     tc.tile_pool(name=f"st{layer}", bufs=2) as stpool, \
                     tc.tile_pool(name=f"s{layer}", bufs=6) as spool, \
                     tc.tile_pool(name=f"ep{layer}", bufs=2) as epool, \
                     tc.tile_pool(name=f"pm{layer}", bufs=2, space="PSUM") as pmpool, \
                     tc.tile_pool(name=f"pd{layer}", bufs=2, space="PSUM") as pdpool, \
                     tc.tile_pool(name=f"pt{layer}", bufs=2, space="PSUM") as ptpool:
                    if "no_edge" in DEBUG_FLAGS:
                        nc.vector.memset(h1T[:], 0.1)
                        if layer == 1:
                            Sz = spool.tile([P, P], dt.bfloat16, name="Sz")
                            nc.vector.memset(Sz[:], 0.0)
                            hz = epool.tile([P, C], dt.bfloat16, name="hz")
                            nc.vector.memset(hz[:], 0.0)
                            nc.tensor.matmul(ppool0[:], Sz[:], hz[:], start=True, stop=True)
                            nc.tensor.matmul(ppool1[:], Sz[:], hz[:], start=True, stop=True)
                        return
                    for sb in range(NSB):
                        ilo = stpool.tile([P, SBT * CLn * 8], dt.int16, name="ilo")
                        nc.sync.dma_start(out=ilo[:], in_=idxlo_in[sb])
                        ihi = stpool.tile([P, SBT * CHn * 8], dt.int16, name="ihi")
                        nc.sync.dma_start(out=ihi[:], in_=idxhi_in[sb])
                        dl_sb = stpool.tile([P, M], dt.float32, name="dl_sb")
                        nc.sync.dma_start(out=dl_sb[:], in_=dstloc_in[sb])
                        adi = stpool.tile([P, M * 8], dt.int16, name="adi")
                        nc.sync.dma_start(out=adi[:], in_=adidx_in[sb])

                        gbuf = gpool.tile([P, M, HW_], dt.bfloat16, name="gbuf")
                        adg = gpool.tile([P, M, 128], dt.bfloat16, name="adg")
                        if "no_gather" in DEBUG_FLAGS:
                            nc.vector.memset(gbuf[:].rearrange("p a b -> p (a b)"), 0.25)
                            nc.vector.memset(adg[:].rearrange("p a b -> p (a b)"), 0.1)
                        elif "no_adg" in DEBUG_FLAGS:
                            nc.vector.memset(adg[:].rearrange("p a b -> p (a b)"), 0.1)
                            nc.gpsimd.dma_gather(
                                gbuf[:, 0:CLg, :], h_full[layer][:], ilo[:],
                                num_idxs=CLg * P, num_idxs_reg=CLg * P, elem_size=HW_,
                                single_packet=False, queue_num=0)
                            nc.gpsimd.dma_gather(
                                gbuf[:, CLg:M, :], h_full[layer][SPLIT:, :], ihi[:],
                                num_idxs=SBT * CHn * P, num_idxs_reg=SBT * CHn * P,
                                elem_size=HW_, single_packet=False, queue_num=1)
                        else:
                            nc.gpsimd.dma_gather(
                                gbuf[:, 0:CLg, :], h_full[layer][:], ilo[:],
                                num_idxs=CLg * P, num_idxs_reg=CLg * P, elem_size=HW_,
                                single_packet=False, queue_num=0)
                            nc.gpsimd.dma_gather(
                                gbuf[:, CLg:M, :], h_full[layer][SPLIT:, :], ihi[:],
                                num_idxs=SBT * CHn * P, num_idxs_reg=SBT * CHn * P,
                                elem_size=HW_, single_packet=False, queue_num=1)
                            nc.gpsimd.dma_gather(
                                adg[:], ad_tab[layer][:], adi[:],
                                num_idxs=M * P, num_idxs_reg=M * P, elem_size=128,
                                single_packet=False, queue_num=2)

                        as_v = gbuf[:, :, C:C + 2].bitcast(dt.float32).rearrange(
                            "p m o -> p (m o)")
                        ad_v = adg[:, :, 0]
                        e_t = stpool.tile([P, M], dt.float32, name="e_t")
                        nc.vector.tensor_tensor(out=e_t[:], in0=as_v, in1=ad_v, op=op.add)
                        l_t = stpool.tile([P, M], dt.float32, name="l_t")
                        nc.scalar.activation(l_t[:], e_t[:], act.Prelu, alpha=0.2)
                        ex_t = stpool.tile([P, M], dt.float32, name="ex_t")
                        nc.scalar.activation(ex_t[:], l_t[:], act.Exp)

                        for j in range(SBT):
                            t = sb * SBT + j
                            chunks = ([j * CLn + k for k in range(CLn)] +
                                      [CLg + j * CHn + k for k in range(CHn)])
                            pm = pmpool.tile([P, C], dt.float32, space="PSUM", name="pm")
                            pd = pdpool.tile([P, 8], dt.float32, space="PSUM", name="pd")
                            for ci, n in enumerate(chunks):
                                S = spool.tile([P, P], dt.bfloat16, name="S")
                                nc.vector.tensor_scalar(
                                    S[:], iota_sb[:], dl_sb[:, n:n + 1], ex_t[:, n:n + 1],
                                    op.is_equal, op.mult)
                                nc.tensor.matmul(pm[:], S[:], gbuf[:, n, 0:C],
                                                 start=(ci == 0), stop=(ci == CT - 1))
                                nc.tensor.matmul(pd[:, 0:1], S[:], ones_col[:],
                                                 start=(ci == 0), stop=(ci == CT - 1))
                            if t >= TPC:
                                continue
                            # epilogue
                            dcl = epool.tile([P, 1], dt.float32, name="dcl")
                            nc.vector.tensor_scalar(dcl[:], pd[:, 0:1], 1e-30, None, op.max)
                            rec = epool.tile([P, 1], dt.float32, name="rec")
                            nc.vector.reciprocal(rec[:], dcl[:])
                            osb = epool.tile([P, C], dt.float32, name="osb")
                            nc.vector.tensor_scalar(osb[:], pm[:], rec[:], None, op.mult)
                            xsb = epool.tile([P, C], dt.float32, name="xsb")
                            nc.vector.tensor_tensor(
                                out=xsb[:], in0=osb[:],
                                in1=(b1_bc if layer == 0 else b2_bc)[:], op=op.add)
                            hout = selu_into(epool, xsb[:], "selu_e")
                            if layer == 0:
                                for k in range(2):
                                    ptr = ptpool.tile([P, P], dt.float32, space="PSUM", name="ptr")
                                    nc.tensor.transpose(
                                        ptr[:], hout[:, k * 128:(k + 1) * 128], ident[:])
                                    nc.vector.tensor_copy(
                                        h1T[:, k * NSHP + t * 128:k * NSHP + (t + 1) * 128],
                                        ptr[:])
                            else:
                                hb2 = epool.tile([P, C], dt.bfloat16, name="hb2")
                                nc.vector.tensor_copy(hb2[:], hout[:])
                                g01 = epool.tile([P, 2], dt.float32, name="g01")
                                nc.sync.dma_start(
                                    out=g01[:], in_=g01_in[t * 128:(t + 1) * 128, :])
                                for b_ in range(2):
                                    S0 = spool.tile([P, P], dt.bfloat16, name="S0")
                                    nc.vector.tensor_scalar(
                                        S0[:], iota_sb[:], g01[:, b_:b_ + 1], None, op.is_equal)
                                    nc.tensor.matmul(
                                        (ppool0 if b_ == 0 else ppool1)[:], S0[:], hb2[:],
                                        start=(t == 0), stop=(t == TPC - 1))

            # ================= pipeline =================
            node_phase(0)
            nc.gpsimd.collective_compute(
                "AllGather", mybir.AluOpType.bypass, replica_groups=rg,
                ins=[h_sh[0][:]], outs=[h_full[0][:]])
            edge_phase(0)
            node_phase(1)
            nc.gpsimd.collective_compute(
                "AllGather", mybir.AluOpType.bypass, replica_groups=rg,
                ins=[h_sh[1][:]], outs=[h_full[1][:]])

            with tc.tile_pool(name="poolp", bufs=1, space="PSUM") as plpool:
                ppool0 = plpool.tile([P, C], dt.float32, space="PSUM", name="ppool0")
                ppool1 = plpool.tile([P, C], dt.float32, space="PSUM", name="ppool1")
                edge_phase(1, ppool0, ppool1)

                # ------------- head -------------
                with tc.tile_pool(name="head", bufs=1) as hp, \
                     tc.tile_pool(name="hps", bufs=3, space="PSUM") as hps:
                    pool_sb = hp.tile([P, 2 * C], dt.float32)
                    nc.vector.tensor_copy(pool_sb[:, 0:C], ppool0[:])
                    nc.vector.tensor_copy(pool_sb[:, C:2 * C], ppool1[:])
                    nc.sync.dma_start(out=ppart[0:128, :], in_=pool_sb[:, 0:C])
                    nc.sync.dma_start(out=ppart[128:256, :], in_=pool_sb[:, C:2 * C])
                    nc.gpsimd.collective_compute(
                        "AllReduce", mybir.AluOpType.add, replica_groups=rg,
                        ins=[ppart[:]], outs=[psum_g[:]])
                    for b_ in range(2):
                        gs = hp.tile([P, C], dt.float32, name=f"gs{b_}")
                        nc.sync.dma_start(out=gs[:], in_=psum_g[b_ * 128:(b_ + 1) * 128, :])
                        gm = hp.tile([P, C], dt.float32, name=f"gm{b_}")
                        nc.vector.tensor_scalar(gm[:], gs[:], rcnt_sb[:, b_:b_ + 1], None, op.mult)
                        gsel = selu_into(hp, gm[:], f"selu_g{b_}")
                        # fc1: transpose gsel -> 2 chunks, matmul with fc1w
                        pf1 = hps.tile([P, HID], dt.float32, space="PSUM", name=f"pf1_{b_}", tag="hpsum")
                        gTs = []
                        for k in range(2):
                            ptr = hps.tile([P, P], dt.float32, space="PSUM", name=f"gT_ps{b_}{k}", tag="hpsum")
                            nc.tensor.transpose(ptr[:], gsel[:, k * 128:(k + 1) * 128], ident[:])
                            gT = hp.tile([P, P], dt.float32, name=f"gT{b_}{k}")
                            nc.vector.tensor_copy(gT[:], ptr[:])
                            gTs.append(gT)
                        for k in range(2):
                            nc.tensor.matmul(pf1[:], gTs[k][:], fc1w_sb[k][:],
                                             start=(k == 0), stop=(k == 1))
                        x1 = hp.tile([P, HID], dt.float32, name=f"x1_{b_}")
                        nc.vector.tensor_tensor(out=x1[:], in0=pf1[:], in1=fc1b_bc[:], op=op.add)
                        g1 = selu_into(hp, x1[:], f"selu_f{b_}")
                        # fc2
                        ptr2 = hps.tile([P, P], dt.float32, space="PSUM", name=f"g1T_ps{b_}", tag="hpsum")
                        nc.tensor.transpose(ptr2[:], g1[:], ident[:])
                        g1T = hp.tile([P, P], dt.float32, name=f"g1T{b_}")
                        nc.vector.tensor_copy(g1T[:], ptr2[:])
                        pf2 = hps.tile([P, 8], dt.float32, space="PSUM", name=f"pf2_{b_}", tag="hpsum")
                        nc.tensor.matmul(pf2[:, 0:NCLS], g1T[:], fc2w_sb[:], start=True, stop=True)
                        x2 = hp.tile([P, NCLS], dt.float32, name=f"x2_{b_}")
                        nc.vector.tensor_tensor(out=x2[:], in0=pf2[:, 0:NCLS], in1=fc2b_bc[:], op=op.add)
                        # log_softmax
                        mx = hp.tile([P, 1], dt.float32, name=f"mx{b_}")
                        nc.vector.tensor_reduce(mx[:], x2[:], axis=mybir.AxisListType.X, op=op.max)
                        zc = hp.tile([P, NCLS], dt.float32, name=f"zc{b_}")
                        nc.vector.tensor_scalar(zc[:], x2[:], mx[:], None, op.subtract)
                        ee = hp.tile([P, NCLS], dt.float32, name=f"ee{b_}")
                        nc.scalar.activation(ee[:], zc[:], act.Exp)
                        ssum = hp.tile([P, 1], dt.float32, name=f"ss{b_}")
                        nc.vector.tensor_reduce(ssum[:], ee[:], axis=mybir.AxisListType.X, op=op.add)
                        lls = hp.tile([P, 1], dt.float32, name=f"ll{b_}")
                        nc.scalar.activation(lls[:], ssum[:], act.Ln)
                        oo = hp.tile([P, NCLS], dt.float32, name=f"oo{b_}")
                        nc.vector.tensor_scalar(oo[:], zc[:], lls[:], None, op.subtract)
                        nc.sync.dma_start(out=out_t[b_ * 128:(b_ + 1) * 128, :], in_=oo[:])

    nc.compile()
    return nc


def kernel(**inputs):
    import concourse.bass  # noqa: F401  (path setup)
    from concourse.bass_utils import run_bass_kernel_spmd

    edge_index = np.asarray(inputs["edge_index"], np.int64)
    ep = _edge_prep(edge_index)
    key = (ep["CL"], ep["CH"])
    if key not in _CACHE:
        _CACHE[key] = _build(*key)
    nc = _CACHE[key]
    in_maps = _make_in_maps(inputs, ep)
    res = run_bass_kernel_spmd(nc, in_maps, core_ids=list(range(W)))
    return np.asarray(res.results[0]["out"], np.float32)


def _make_in_maps(inputs, ep):
    x = np.asarray(inputs["x"], np.float32)
    batch = np.asarray(inputs["batch"], np.int64)
    W1 = np.asarray(inputs["W1"], np.float32)
    W2 = np.asarray(inputs["W2"], np.float32)

    def ext(Wm, al, ar):
        Wm64 = Wm.astype(np.float64)
        return np.concatenate(
            [Wm, (Wm64 @ np.asarray(al, np.float64))[:, None].astype(np.float32),
             (Wm64 @ np.asarray(ar, np.float64))[:, None].astype(np.float32)], axis=1)

    w1ext = ext(W1, inputs["att_l1"], inputs["att_r1"])
    w2ext = ext(W2, inputs["att_l2"], inputs["att_r2"])
    cnt = np.bincount(batch, minlength=G).astype(np.float32)
    rcnt = (1.0 / np.maximum(cnt, 1.0)).reshape(G, 1)
    iota = np.broadcast_to(np.arange(P, dtype=np.float32)[None, :], (P, P)).astype(BF16)

    in_maps = []
    for c in range(W):
        nl = ep["newloc"][c * NSH:(c + 1) * NSH]
        xs = np.zeros((NSHP, F_IN), np.float32)
        xs[nl] = x[c * NSH:(c + 1) * NSH]
        g01 = np.full((NSHP, 2), 1e9, np.float32)
        bsh = batch[c * NSH:(c + 1) * NSH].astype(np.float32)
        g01[nl, 0] = bsh
        g01[nl, 1] = bsh - 128.0
        in_maps.append({
            "xT": np.ascontiguousarray(xs.T),
            "w1ext": w1ext, "w2ext": w2ext,
            "b1r": np.asarray(inputs["b1"], np.float32).reshape(1, C),
            "b2r": np.asarray(inputs["b2"], np.float32).reshape(1, C),
            "fc1w": np.asarray(inputs["fc1_W"], np.float32),
            "fc1b": np.asarray(inputs["fc1_b"], np.float32).reshape(1, HID),
            "fc2w": np.asarray(inputs["fc2_W"], np.float32),
            "fc2b": np.asarray(inputs["fc2_b"], np.float32).reshape(1, NCLS),
            "rcnt": rcnt, "g01": g01, "iota128": iota,
            "idxlo": ep["idxlo"][c], "idxhi": ep["idxhi"][c],
            "dstloc": ep["dstloc"][c], "adidx": ep["adidx"][c],
        })
    return in_maps


def benchmark(inputs, iters=12):
    """Repeated sharded execution with device-resident inputs; returns min ns."""
    import time
    import jax
    from jax.sharding import Mesh, PartitionSpec, NamedSharding
    from jax.experimental.shard_map import shard_map
    from concourse import bass2jax
    import concourse.mybir as mybir
    bass2jax.install_neuronx_cc_hook()

    edge_index = np.asarray(inputs["edge_index"], np.int64)
    ep = _edge_prep(edge_index)
    key = (ep["CL"], ep["CH"])
    if key not in _CACHE:
        _CACHE[key] = _build(*key)
    nc = _CACHE[key]
    in_maps = _make_in_maps(inputs, ep)

    part_name = nc.partition_id_tensor.name if nc.partition_id_tensor else None
    in_names, out_names, out_avals, zero_outs = [], [], [], []
    for alloc in nc.m.functions[0].allocations:
        if not isinstance(alloc, mybir.MemoryLocationSet):
            continue
        name = alloc.memorylocations[0].name
        if alloc.kind == "ExternalInput":
            if name != part_name:
                in_names.append(name)
        elif alloc.kind == "ExternalOutput":
            out_names.append(name)
            shape = tuple(alloc.tensor_shape)
            dtype = mybir.dt.np(alloc.dtype)
            out_avals.append(jax.core.ShapedArray(shape, dtype))
            zero_outs.append(np.zeros(shape, dtype))
    n_params = len(in_names)
    all_in = in_names + out_names
    if part_name is not None:
        all_in = all_in + [part_name]

    def _body(*args):
        operands = list(args)
        if part_name is not None:
            operands.append(bass2jax.partition_id_tensor())
        outs = bass2jax._bass_exec_p.bind(
            *operands, out_avals=tuple(out_avals), in_names=tuple(all_in),
            out_names=tuple(out_names), lowering_input_output_aliases=(),
            sim_require_finite=True, sim_require_nnan=True, nc=nc)
        return tuple(outs)

    devices = jax.devices()[:W]
    mesh = Mesh(np.asarray(devices), ("core",))
    nin = n_params + len(out_names)
    f1 = jax.jit(shard_map(_body, mesh=mesh, in_specs=(PartitionSpec("core"),) * nin,
                           out_specs=(PartitionSpec("core"),) * len(out_names),
                           check_rep=False), keep_unused=True)
    concat_in = [np.concatenate([in_maps[c][nm] for c in range(W)], axis=0)
                 for nm in in_names]
    concat_zero = [np.zeros((W * z.shape[0], *z.shape[1:]), z.dtype) for z in zero_outs]
    sh = NamedSharding(mesh, PartitionSpec("core"))
    dev_args = [jax.device_put(a, sh) for a in concat_in + concat_zero]
    jax.block_until_ready(f1(*dev_args))

    # async pipelined submissions: device executions serialize per-core while
    # dispatch overlaps, so (t_N - t_1)/(N-1) isolates per-execution time.
    def run_n(n):
        ts = []
        for _ in range(iters):
            t0 = time.perf_counter()
            r = None
            for _ in range(n):
                r = f1(*dev_args)
            jax.block_until_ready(r)
            ts.append(time.perf_counter() - t0)
        ts.sort()
        return ts[0]

    NCH = 16
    t1 = run_n(1)
    tN = run_n(NCH)
    per_exec = (tN - t1) / (NCH - 1)
    print(f"  t1 min={t1*1e3:.3f}ms | t{NCH} min={tN*1e3:.3f}ms -> per-exec {per_exec*1e6:.1f}us")
    return per_exec * 1e9


if __name__ == "__main__":
    import time
    ei = np.load("/tmp/edge_index.npy")
    t0 = time.time()
    ep = _edge_prep(ei)
    print("edge prep:", round(time.time() - t0, 1), "s; CL/CH:", ep["CL"], ep["CH"])
